# revision 40
# baseline (speedup 1.0000x reference)
"""Multi-head attention (B=4, N=2048, DIM=1024, H=16, DH=64) on 8 trn2 cores.

Sharding: core c handles batch c//2 and head-half c%2 (8 heads).  Each core
computes qkv projection for its heads, attention, and a partial output
projection; the host sums the two partials per batch and adds the bias.
No cross-core collectives needed.

Layout strategy (zero on-device transposes):
  - host supplies x[b] pre-transposed (xT: [DIM, N]) in bf16
  - qT/kT computed as [d, n] ("transposed") via out = W^T @ x^T matmuls
  - S^T tiles [j=128, i=512] from row-packed matmuls (d=64 contraction,
    2 heads concurrently in PE row groups 0-63 / 64-127)
  - exp via ACT (scale folded), PSUM -> SBUF bf16 (P^T tiles)
  - PV: O^T[d, i] += V[j, d]^T-matmul, col-packed pairs (PE col groups)
  - denominators: DVE add-tree over j-tiles, then a broadcast-ones matmul
    (lhsT = ones[128, 64]) accumulating lo+hi roots in PSUM so the
    per-head denominator lands replicated across the 64 partitions that
    match oT_ps's head rows -- no gpsimd broadcast, single recip+normalize
  - the PAR tail (dnr matmuls, recip, normalize) is deferred into the
    next unit's j-loop so PE's in-order queue never blocks the next
    unit's S matmuls
  - prologue is minimal (q-ib0, k-ib0, v0); the rest of pair-0's k/q and
    all remaining V projections dribble into unit 0 as extras
  - output projection consumes O^T tiles directly as lhsT
"""

import numpy as np
import ml_dtypes

B, N, DIM = 4, 2048, 1024
HEADS, DH = 16, 64
SCALE = DIM ** (-0.5)
HPC = 8              # heads per core
NPAIR = HPC // 2     # 4 head pairs
CPC = HPC * DH       # 512 channels per core
IB = 512             # i-block (query cols per attention unit)
NIB = N // IB        # 4
NJT = N // 128       # 16 j-tiles
NKT = DIM // 128     # 8 contraction tiles for projections

_cache = {}


def _build():
    import concourse.bacc as bacc
    import concourse.mybir as mybir
    import concourse.tile as tile

    f32 = mybir.dt.float32
    bf16 = mybir.dt.bfloat16

    nc = bacc.Bacc("TRN2", target_bir_lowering=False, debug=False,
                   enable_asserts=False, num_devices=8)

    xT_d = nc.dram_tensor("xT", (DIM, N), bf16, kind="ExternalInput").ap()
    wqkv_d = nc.dram_tensor("wqkv", (DIM, 3 * CPC), bf16, kind="ExternalInput").ap()
    wout_d = nc.dram_tensor("wout", (CPC, DIM), bf16, kind="ExternalInput").ap()
    out_d = nc.dram_tensor("out", (N, DIM), bf16, kind="ExternalOutput").ap()

    with tile.TileContext(nc) as tc:
        _body(nc, tc, mybir, xT_d, wqkv_d, wout_d, out_d)

    nc.compile()
    return nc


def _body(nc, tc, mybir, xT_d, wqkv_d, wout_d, out_d):
    from contextlib import ExitStack

    f32 = mybir.dt.float32
    bf16 = mybir.dt.bfloat16
    Exp = mybir.ActivationFunctionType.Exp
    mult = mybir.AluOpType.mult
    add = mybir.AluOpType.add
    NJH = NJT // 2   # j-tiles per half (8)

    ctx = ExitStack()
    with ctx:
        wpool = ctx.enter_context(tc.tile_pool(name="weights", bufs=1))
        qkv_pool = ctx.enter_context(tc.tile_pool(name="qkv", bufs=1))
        ppool = ctx.enter_context(tc.tile_pool(name="ptiles", bufs=2))
        ppool1 = ctx.enter_context(tc.tile_pool(name="ptiles1", bufs=1))
        spool = ctx.enter_context(tc.tile_pool(name="small", bufs=2))
        outp = ctx.enter_context(tc.tile_pool(name="outstage", bufs=3))
        opool = ctx.enter_context(tc.tile_pool(name="oT", bufs=16))
        psum = ctx.enter_context(tc.tile_pool(name="psum", bufs=2, space="PSUM"))

        # ---- weights + xT staged loads: x/wqk first so compute starts
        # early; v columns and wout follow ----
        wqkv_sb = wpool.tile([128, NKT, 3 * CPC], bf16)
        wqkv_r = wqkv_d.rearrange("(ko p) c -> p ko c", p=128)
        xT_sb = wpool.tile([128, NKT, N], bf16)
        xT_r = xT_d.rearrange("(ko p) n -> p ko n", p=128)
        # few large DMAs (per-op fixed cost dominates small ones), ordered by
        # first use: pair-0 q/k weight columns, then x in i/j column chunks
        # (each chunk unlocks q/k-ib and v(jt) projections for that range),
        # v columns early, remaining q/k weight columns after
        nc.sync.dma_start(wqkv_sb[:, :, 0:128], wqkv_r[:, :, 0:128])
        nc.sync.dma_start(wqkv_sb[:, :, CPC:CPC + 128],
                          wqkv_r[:, :, CPC:CPC + 128])
        nc.sync.dma_start(xT_sb[:, :, 0:512], xT_r[:, :, 0:512])
        nc.sync.dma_start(wqkv_sb[:, :, 2 * CPC:3 * CPC],
                          wqkv_r[:, :, 2 * CPC:3 * CPC])
        nc.sync.dma_start(xT_sb[:, :, 512:1024], xT_r[:, :, 512:1024])
        nc.sync.dma_start(xT_sb[:, :, 1024:2048], xT_r[:, :, 1024:2048])
        nc.sync.dma_start(wqkv_sb[:, :, 128:CPC], wqkv_r[:, :, 128:CPC])
        nc.sync.dma_start(wqkv_sb[:, :, CPC + 128:2 * CPC],
                          wqkv_r[:, :, CPC + 128:2 * CPC])
        wout_sb = wpool.tile([128, NPAIR, DIM], bf16)
        nc.sync.dma_start(wout_sb, wout_d.rearrange("(po p) n -> p po n", p=128))

        # all-ones: cols 0:64 serve the broadcast denominator matmuls; the
        # full [128, 512] doubles as the PE warmup operand
        ones_sb = wpool.tile([128, 512], bf16)
        nc.gpsimd.memset(ones_sb, 1.0)

        # PE pstate warmup during the DMA wait: ~4us of N=512 matmuls so the
        # first real projection bursts start at full clock (results unused)
        for w in range(8):
            wps = psum.tile([64, 512], f32, tag="qkvps", name="warm")
            nc.tensor.matmul(wps, lhsT=ones_sb[:, 0:64], rhs=ones_sb,
                             start=True, stop=True)

        # per-pair q/k tiles (separate tiles => clean dependency tracking
        # when later pairs' projections interleave into attention units)
        qT_t = [qkv_pool.tile([128, N], bf16, tag=f"qT{p}", name=f"qT{p}") for p in range(NPAIR)]
        kT_t = [qkv_pool.tile([128, N], bf16, tag=f"kT{p}", name=f"kT{p}") for p in range(NPAIR)]
        v_sb = qkv_pool.tile([128, NJT, CPC], bf16)

        # ---- emit helpers ----
        def qk_steps_one(p, qk, ib):
            """Two-burst projection of q or k for pair p, i-block ib."""
            woff = qk * CPC + p * 128
            dst = (qT_t if qk == 0 else kT_t)[p]
            cell = {}

            def stepA(cell=cell, woff=woff, ib=ib):
                cell["ps"] = psum.tile([128, IB], f32, tag="qkvps", name="qkps")
                for kt in range(4):
                    nc.tensor.matmul(
                        cell["ps"],
                        lhsT=wqkv_sb[:, kt, woff:woff + 128],
                        rhs=xT_sb[:, kt, ib * IB:(ib + 1) * IB],
                        start=(kt == 0), stop=False)

            def stepB(cell=cell, woff=woff, ib=ib, dst=dst):
                for kt in range(4, NKT):
                    nc.tensor.matmul(
                        cell["ps"],
                        lhsT=wqkv_sb[:, kt, woff:woff + 128],
                        rhs=xT_sb[:, kt, ib * IB:(ib + 1) * IB],
                        start=False, stop=(kt == NKT - 1))
                nc.vector.tensor_copy(
                    out=dst[:, ib * IB:(ib + 1) * IB], in_=cell["ps"])

            return [stepA, stepB]

        def qk_steps(p):
            steps = []
            for qk in (0, 1):
                for ib in range(NIB):
                    steps += qk_steps_one(p, qk, ib)
            return steps

        def emit_v(jt):
            ps = psum.tile([128, CPC], f32, tag="qkvps")
            for kt in range(NKT):
                nc.tensor.matmul(
                    ps,
                    lhsT=xT_sb[:, kt, jt * 128:(jt + 1) * 128],
                    rhs=wqkv_sb[:, kt, 2 * CPC:3 * CPC],
                    start=(kt == 0), stop=(kt == NKT - 1))
            nc.vector.tensor_copy(out=v_sb[:, jt, :], in_=ps)

        oT_all = {}

        def outproj_steps(ib, tags=("qkvps",)):
            steps = []
            for isub in range(4):
                for nh in range(2):
                    tg = tags[(isub * 2 + nh) % len(tags)]
                    def step(isub=isub, nh=nh, ib=ib, tg=tg):
                        ops = psum.tile([128, 512], f32, tag=tg)
                        for p in range(NPAIR):
                            nc.tensor.matmul(
                                ops,
                                lhsT=oT_all[(p, ib)][:, isub * 128:(isub + 1) * 128],
                                rhs=wout_sb[:, p, nh * 512:(nh + 1) * 512],
                                start=(p == 0), stop=(p == NPAIR - 1))
                        # bf16 staging: 2x-mode DVE copy and half the DMA bytes
                        ost = outp.tile([128, 512], bf16, tag="ost")
                        nc.vector.tensor_copy(out=ost, in_=ops)
                        nc.sync.dma_start(
                            out_d[ib * IB + isub * 128: ib * IB + (isub + 1) * 128,
                                  nh * 512:(nh + 1) * 512], ost)
                    steps.append(step)
            return steps

        # ---- attention unit ----
        # P^T for a unit lives in two half tiles (j-tiles 0-7 / 8-15), each
        # [128, 2*NJH, IB] bf16 with planes indexed 2*jt_local + head.
        # The denominator add-tree runs in-place per half; roots end at
        # plane pair 0 (jt0 for lo, jt8 for hi).
        def emit_S(p, ib, jt, sAB, lo_t, hi_t):
            isl_ = slice(ib * IB, (ib + 1) * IB)
            jsl = slice(jt * 128, (jt + 1) * 128)
            t, j = (lo_t, jt) if jt < NJH else (hi_t, jt - NJH)
            nc.tensor.matmul(
                sAB[:, 0:IB],
                lhsT=kT_t[p][0:64, jsl],
                rhs=qT_t[p][0:64, isl_],
                start=True, stop=True, tile_position=(0, 0))
            nc.tensor.matmul(
                sAB[:, IB:2 * IB],
                lhsT=kT_t[p][64:128, jsl],
                rhs=qT_t[p][64:128, isl_],
                start=True, stop=True, tile_position=(64, 0))
            nc.scalar.activation(
                t[:, 2 * j:2 * j + 2, :].rearrange("p a b -> p (a b)"),
                sAB, Exp, scale=SCALE)

        def unit(p, ib, extras, handoff=None, next_info=None):
            # handoff: (lo_tile, npre) -- S/exp for jt < npre were already
            # emitted by the previous unit (cross-unit ACT backfill)
            if handoff is None:
                lo = ppool.tile([128, 2 * NJH, IB], bf16, tag="ptlo")
                npre = 0
            else:
                lo, npre = handoff
            hi = ppool1.tile([128, 2 * NJH, IB], bf16, tag="pthi")
            oT_ps = psum.tile([128, IB], f32, tag="oT")

            def pthalf(jt):
                return (lo, jt) if jt < NJH else (hi, jt - NJH)

            def emit_pv(jt):
                t, j = pthalf(jt)
                st = (jt == 0)
                sp = (jt == NJT - 1)
                nc.tensor.matmul(
                    oT_ps[0:64, :],
                    lhsT=v_sb[:, jt, (2 * p) * DH:(2 * p + 1) * DH],
                    rhs=t[:, 2 * j, :],
                    start=st, stop=sp, tile_position=(0, 0))
                nc.tensor.matmul(
                    oT_ps[64:128, :],
                    lhsT=v_sb[:, jt, (2 * p + 1) * DH:(2 * p + 2) * DH],
                    rhs=t[:, 2 * j + 1, :],
                    start=st, stop=sp, tile_position=(0, 64))

            # paced balanced merge tree over j for the denominators: one
            # in-place [128, 1024] bf16 add per iter (depth-4, same
            # accuracy as a bulk tree, but no end-of-unit DVE burst).
            def pl(t, a, n=2):
                return t[:, a:a + n, :].rearrange("p a b -> p (a b)")

            def merge(jt_dst, jt_src):
                td, d = pthalf(jt_dst)
                ts_, sr = pthalf(jt_src)
                nc.vector.tensor_tensor(
                    pl(td, 2 * d), pl(td, 2 * d), pl(ts_, 2 * sr), add)

            msched = {2: [(0, 1)], 4: [(2, 3)], 6: [(4, 5)], 8: [(6, 7)],
                      9: [(0, 2)], 10: [(8, 9)], 11: [(4, 6)], 12: [(10, 11)],
                      13: [(0, 4)], 14: [(8, 10), (12, 13)], 15: [(14, 15)]}
            extras = dict(extras)
            # with a deep backfill (npre=8), emit the remaining own S tiles
            # early (iters 2-9) so ACT never drains; not before iter 2 --
            # iter 1's deferred dnr must read the prior unit's hi planes
            # before this unit's hi tile is first written
            shift = max(0, npre - 2)
            for jt in range(NJT):
                jts = jt + shift
                if npre <= jts < NJT:
                    sAB = psum.tile([128, 2 * IB], f32, tag="sAB")
                    emit_S(p, ib, jts, sAB, lo, hi)
                if jt >= 1:
                    emit_pv(jt - 1)
                for fn in extras.pop(jt, ()):
                    fn()
                for (a, b) in msched.get(jt, ()):
                    merge(a, b)
            nxt_lo = None
            if next_info is not None:
                np_, nib = next_info
                nxt_lo = ppool.tile([128, 2 * NJH, IB], bf16, tag="ptlo",
                                    name="ptlo_h")
                sAB_h = psum.tile([128, 2 * IB], f32, tag="sAB", name="sAB_h")
                emit_S(np_, nib, 0, sAB_h, nxt_lo, None)
            emit_pv(NJT - 1)
            merge(12, 14)
            merge(8, 12)

            # PAR tail, deferred into the next unit's j-loop: the broadcast
            # denominator matmuls (PE) accumulate lo-root + hi-root per head
            # into dn_ps with the per-head sum replicated on 64 partitions;
            # recip + single normalize follow on DVE.
            def tail_dnr(p=p, ib=ib, lo=lo, hi=hi, oT_ps=oT_ps):
                cell = {}

                def dnr_mm(cell=cell):
                    dn_ps = psum.tile([128, IB], f32, tag="qkvps", name="dnps")
                    for h in range(2):
                        nc.tensor.matmul(
                            dn_ps[h * 64:(h + 1) * 64, :],
                            lhsT=ones_sb[:, 0:64], rhs=lo[:, h, :],
                            start=True, stop=False, tile_position=(0, h * 64))
                        nc.tensor.matmul(
                            dn_ps[h * 64:(h + 1) * 64, :],
                            lhsT=ones_sb[:, 0:64], rhs=hi[:, h, :],
                            start=False, stop=True, tile_position=(0, h * 64))
                    cell["dn_ps"] = dn_ps

                def norm(cell=cell):
                    dn = spool.tile([128, IB], f32, tag="dn")
                    nc.vector.reciprocal_approx_fast(dn, cell["dn_ps"])
                    oT_sb = opool.tile([128, IB], bf16, tag="oTsb")
                    nc.vector.tensor_tensor(oT_sb, oT_ps, dn, mult)
                    oT_all[(p, ib)] = oT_sb

                return [dnr_mm, norm]

            return (nxt_lo, 1) if nxt_lo is not None else None, tail_dnr()

        # ---- minimal prologue: q-ib0, k-ib0 for pair 0 ----
        for st in qk_steps_one(0, 0, 0):
            st()
        for st in qk_steps_one(0, 1, 0):
            st()

        # ---- main sweep: pair-outer / i-block-inner ----
        # extras injected per unit:
        #   unit 0: remaining V tiles (lag-1 ahead of PV), k-ib1..3 in time
        #           for S at jt=4*ib, q-ib1 late (needed by unit 1)
        #   units 1-2: pair-0's remaining q projections
        #   p<3 units 1..3 of each pair phase: next pair's qk bursts
        #   p==3 units: previous i-block's output projection
        #   every unit: previous unit's deferred PAR tail at jt=1,2
        seq = [(p, ib) for p in range(NPAIR) for ib in range(NIB)]
        # lo tiles for units 0 and 1 pre-allocated in rotation order so the
        # cross-unit S backfill (unit 0 emits unit 1's jt 0-7) keeps the
        # ppool slot-reuse ordering safe.  NOTE: this backfill must stay a
        # one-shot for unit 0 -- a STANDING npre=8 pipeline was measured
        # 38us SLOWER because the permanent ACT backlog keeps both sAB psum
        # slots full and the in-order PE head-of-line blocks on them.
        u0_lo = ppool.tile([128, 2 * NJH, IB], bf16, tag="ptlo", name="ptlo0")
        u1_lo = ppool.tile([128, 2 * NJH, IB], bf16, tag="ptlo", name="ptlo1")
        handoff = (u0_lo, 0)
        tail = None
        for i, (p, ib) in enumerate(seq):
            extras = {}
            if i == 0:
                # v(jt) one step ahead of its PV; k-ib blocks before their
                # first S use; q-ib1 early so unit 1's S tiles can backfill
                # ACT's idle windows here (npre=8 handoff)
                extras[0] = [lambda: emit_v(0)]
                for jt in range(1, NJT):
                    extras.setdefault(jt, []).append(lambda jt=jt: emit_v(jt))
                for pos, st in zip((2, 3), qk_steps_one(0, 1, 1)):
                    extras.setdefault(pos, []).append(st)
                for pos, st in zip((3, 4), qk_steps_one(0, 0, 1)):
                    extras.setdefault(pos, []).append(st)
                for pos, st in zip((6, 7), qk_steps_one(0, 1, 2)):
                    extras.setdefault(pos, []).append(st)
                for pos, st in zip((10, 11), qk_steps_one(0, 1, 3)):
                    extras.setdefault(pos, []).append(st)
                for jp in range(8):
                    def pre(jp=jp):
                        sABp = psum.tile([128, 2 * IB], f32, tag="sAB",
                                         name="sAB_h")
                        emit_S(0, 1, jp, sABp, u1_lo, None)
                    extras.setdefault(4 + jp, []).append(pre)
            else:
                spread = []
                if i == 1:
                    spread += qk_steps_one(0, 0, 2) + qk_steps_one(0, 0, 3)
                if p + 1 < NPAIR and ib == 2:
                    spread += qk_steps(p + 1)[0:6]
                elif p + 1 < NPAIR and ib == 3:
                    spread += qk_steps(p + 1)[6:12]
                elif p >= 1 and ib == 0:
                    spread += qk_steps(p)[12:16]
                elif p == 3 and ib >= 1:
                    # front-loaded (>=3: pos 2's deferred tail produces the
                    # oT operand) so the unit's last exps run gap-free and
                    # the epilogue's norm gate clears promptly
                    spread += outproj_steps(ib - 1)
                # one step per position from jt=3, overflow doubles up late
                positions = list(range(3, NJT)) + list(range(8, NJT))
                if p == 3:
                    positions = list(range(3, 11))
                for pos, st in zip(positions, spread):
                    extras.setdefault(pos, []).append(st)
            if tail is not None:
                for pos, st in zip((1, 2), tail):
                    extras.setdefault(pos, []).append(st)
            nxt_info = seq[i + 1] if i + 1 < len(seq) else None
            if i == 0:
                nxt_info = None  # unit 1's jt 0-7 already backfilled
            handoff, tail = unit(p, ib, extras, handoff=handoff,
                                 next_info=nxt_info)
            if i == 0:
                handoff = (u1_lo, 8)
        # epilogue: bridge the norm window -- the first outproj chain's
        # pair-0..2 matmuls don't depend on the final normalize, so emit
        # them between dnr and norm to keep PE dense (and its pstate up)
        tail[0]()
        fib = NIB - 1
        ops0 = psum.tile([128, 512], f32, tag="qkvps", name="op0")
        for p in range(NPAIR - 1):
            nc.tensor.matmul(
                ops0, lhsT=oT_all[(p, fib)][:, 0:128],
                rhs=wout_sb[:, p, 0:512], start=(p == 0), stop=False)
        tail[1]()
        nc.tensor.matmul(
            ops0, lhsT=oT_all[(3, fib)][:, 0:128],
            rhs=wout_sb[:, 3, 0:512], start=False, stop=True)
        ost0 = outp.tile([128, 512], bf16, tag="ost")
        nc.vector.tensor_copy(out=ost0, in_=ops0)
        nc.sync.dma_start(out_d[fib * IB:fib * IB + 128, 0:512], ost0)
        # sAB psum banks are free after the last exp: alternate tags so four
        # chains are in flight and PE stays dense through the tail
        for st in outproj_steps(fib, tags=("sAB", "qkvps"))[1:]:
            st()


def _prep_inputs(x, w_qkv, w_out):
    bf = ml_dtypes.bfloat16
    in_maps = []
    for c in range(8):
        b, hh = c // 2, c % 2
        xT = np.ascontiguousarray(x[b].T).astype(bf)
        q = w_qkv[:, hh * CPC:(hh + 1) * CPC]
        k = w_qkv[:, DIM + hh * CPC: DIM + (hh + 1) * CPC]
        v = w_qkv[:, 2 * DIM + hh * CPC: 2 * DIM + (hh + 1) * CPC]
        wqkv = np.ascontiguousarray(np.concatenate([q, k, v], axis=1)).astype(bf)
        wout = np.ascontiguousarray(w_out[hh * CPC:(hh + 1) * CPC, :]).astype(bf)
        in_maps.append({"xT": xT, "wqkv": wqkv, "wout": wout})
    return in_maps


def _run(x, w_qkv, w_out, b_out, trace=False):
    from concourse import bass_utils
    if "nc" not in _cache:
        _cache["nc"] = _build()
    nc = _cache["nc"]
    in_maps = _prep_inputs(x, w_qkv, w_out)
    res = bass_utils.run_bass_kernel_spmd(
        nc, in_maps, core_ids=list(range(8)), trace=trace)
    partials = [np.asarray(r["out"], dtype=np.float32) for r in res.results]
    out = np.empty((B, N, DIM), dtype=np.float32)
    for b in range(B):
        out[b] = partials[2 * b] + partials[2 * b + 1] + b_out.astype(np.float32)
    return out, res


def kernel(x, w_qkv, w_out, b_out):
    x = np.asarray(x, dtype=np.float32)
    w_qkv = np.asarray(w_qkv, dtype=np.float32)
    w_out = np.asarray(w_out, dtype=np.float32)
    b_out = np.asarray(b_out, dtype=np.float32)
    out, _ = _run(x, w_qkv, w_out, b_out, trace=False)
    return out


# revision 41
# speedup vs baseline: 1.0065x; 1.0065x over previous
"""Multi-head attention (B=4, N=2048, DIM=1024, H=16, DH=64) on 8 trn2 cores.

Sharding: core c handles batch c//2 and head-half c%2 (8 heads).  Each core
computes qkv projection for its heads, attention, and a partial output
projection; the host sums the two partials per batch and adds the bias.
No cross-core collectives needed.

Layout strategy (zero on-device transposes):
  - host supplies x[b] pre-transposed (xT: [DIM, N]) in bf16
  - qT/kT computed as [d, n] ("transposed") via out = W^T @ x^T matmuls
  - S^T tiles [j=128, i=512] from row-packed matmuls (d=64 contraction,
    2 heads concurrently in PE row groups 0-63 / 64-127)
  - exp via ACT (scale folded), PSUM -> SBUF bf16 (P^T tiles)
  - PV: O^T[d, i] += V[j, d]^T-matmul, col-packed pairs (PE col groups)
  - denominators: DVE add-tree over j-tiles, then a broadcast-ones matmul
    (lhsT = ones[128, 64]) accumulating lo+hi roots in PSUM so the
    per-head denominator lands replicated across the 64 partitions that
    match oT_ps's head rows -- no gpsimd broadcast, single recip+normalize
  - the PAR tail (dnr matmuls, recip, normalize) is deferred into the
    next unit's j-loop so PE's in-order queue never blocks the next
    unit's S matmuls
  - prologue is minimal (q-ib0, k-ib0, v0); the rest of pair-0's k/q and
    all remaining V projections dribble into unit 0 as extras
  - output projection consumes O^T tiles directly as lhsT
"""

import numpy as np
import ml_dtypes

B, N, DIM = 4, 2048, 1024
HEADS, DH = 16, 64
SCALE = DIM ** (-0.5)
HPC = 8              # heads per core
NPAIR = HPC // 2     # 4 head pairs
CPC = HPC * DH       # 512 channels per core
IB = 512             # i-block (query cols per attention unit)
NIB = N // IB        # 4
NJT = N // 128       # 16 j-tiles
NKT = DIM // 128     # 8 contraction tiles for projections

_cache = {}


def _build():
    import concourse.bacc as bacc
    import concourse.mybir as mybir
    import concourse.tile as tile

    f32 = mybir.dt.float32
    bf16 = mybir.dt.bfloat16

    nc = bacc.Bacc("TRN2", target_bir_lowering=False, debug=False,
                   enable_asserts=False, num_devices=8)

    xT_d = nc.dram_tensor("xT", (DIM, N), bf16, kind="ExternalInput").ap()
    wqkv_d = nc.dram_tensor("wqkv", (DIM, 3 * CPC), bf16, kind="ExternalInput").ap()
    wout_d = nc.dram_tensor("wout", (CPC, DIM), bf16, kind="ExternalInput").ap()
    out_d = nc.dram_tensor("out", (N, DIM), bf16, kind="ExternalOutput").ap()

    with tile.TileContext(nc) as tc:
        _body(nc, tc, mybir, xT_d, wqkv_d, wout_d, out_d)

    nc.compile()
    return nc


def _body(nc, tc, mybir, xT_d, wqkv_d, wout_d, out_d):
    from contextlib import ExitStack

    f32 = mybir.dt.float32
    bf16 = mybir.dt.bfloat16
    Exp = mybir.ActivationFunctionType.Exp
    mult = mybir.AluOpType.mult
    add = mybir.AluOpType.add
    NJH = NJT // 2   # j-tiles per half (8)

    ctx = ExitStack()
    with ctx:
        wpool = ctx.enter_context(tc.tile_pool(name="weights", bufs=1))
        qkv_pool = ctx.enter_context(tc.tile_pool(name="qkv", bufs=1))
        ppool = ctx.enter_context(tc.tile_pool(name="ptiles", bufs=2))
        ppool1 = ctx.enter_context(tc.tile_pool(name="ptiles1", bufs=1))
        spool = ctx.enter_context(tc.tile_pool(name="small", bufs=2))
        outp = ctx.enter_context(tc.tile_pool(name="outstage", bufs=3))
        opool = ctx.enter_context(tc.tile_pool(name="oT", bufs=16))
        psum = ctx.enter_context(tc.tile_pool(name="psum", bufs=2, space="PSUM"))

        # ---- weights + xT staged loads: x/wqk first so compute starts
        # early; v columns and wout follow ----
        wqkv_sb = wpool.tile([128, NKT, 3 * CPC], bf16)
        wqkv_r = wqkv_d.rearrange("(ko p) c -> p ko c", p=128)
        xT_sb = wpool.tile([128, NKT, N], bf16)
        xT_r = xT_d.rearrange("(ko p) n -> p ko n", p=128)
        # few large DMAs (per-op fixed cost dominates small ones), ordered by
        # first use: pair-0 q/k weight columns, then x in i/j column chunks
        # (each chunk unlocks q/k-ib and v(jt) projections for that range),
        # v columns early, remaining q/k weight columns after
        nc.sync.dma_start(wqkv_sb[:, :, 0:128], wqkv_r[:, :, 0:128])
        nc.sync.dma_start(wqkv_sb[:, :, CPC:CPC + 128],
                          wqkv_r[:, :, CPC:CPC + 128])
        nc.sync.dma_start(xT_sb[:, :, 0:512], xT_r[:, :, 0:512])
        nc.sync.dma_start(wqkv_sb[:, :, 2 * CPC:3 * CPC],
                          wqkv_r[:, :, 2 * CPC:3 * CPC])
        nc.sync.dma_start(xT_sb[:, :, 512:1024], xT_r[:, :, 512:1024])
        nc.sync.dma_start(xT_sb[:, :, 1024:2048], xT_r[:, :, 1024:2048])
        nc.sync.dma_start(wqkv_sb[:, :, 128:CPC], wqkv_r[:, :, 128:CPC])
        nc.sync.dma_start(wqkv_sb[:, :, CPC + 128:2 * CPC],
                          wqkv_r[:, :, CPC + 128:2 * CPC])
        wout_sb = wpool.tile([128, NPAIR, DIM], bf16)
        nc.sync.dma_start(wout_sb, wout_d.rearrange("(po p) n -> p po n", p=128))

        # all-ones: cols 0:64 serve the broadcast denominator matmuls; the
        # full [128, 512] doubles as the PE warmup operand
        ones_sb = wpool.tile([128, 512], bf16)
        nc.gpsimd.memset(ones_sb, 1.0)

        # PE pstate warmup during the DMA wait: ~4us of N=512 matmuls so the
        # first real projection bursts start at full clock (results unused)
        for w in range(8):
            wps = psum.tile([64, 512], f32, tag="qkvps", name="warm")
            nc.tensor.matmul(wps, lhsT=ones_sb[:, 0:64], rhs=ones_sb,
                             start=True, stop=True)

        # per-pair q/k tiles (separate tiles => clean dependency tracking
        # when later pairs' projections interleave into attention units)
        qT_t = [qkv_pool.tile([128, N], bf16, tag=f"qT{p}", name=f"qT{p}") for p in range(NPAIR)]
        kT_t = [qkv_pool.tile([128, N], bf16, tag=f"kT{p}", name=f"kT{p}") for p in range(NPAIR)]
        v_sb = qkv_pool.tile([128, NJT, CPC], bf16)

        # ---- emit helpers ----
        def qk_steps_one(p, qk, ib):
            """Two-burst projection of q or k for pair p, i-block ib."""
            woff = qk * CPC + p * 128
            dst = (qT_t if qk == 0 else kT_t)[p]
            cell = {}

            def stepA(cell=cell, woff=woff, ib=ib):
                cell["ps"] = psum.tile([128, IB], f32, tag="qkvps", name="qkps")
                for kt in range(4):
                    nc.tensor.matmul(
                        cell["ps"],
                        lhsT=wqkv_sb[:, kt, woff:woff + 128],
                        rhs=xT_sb[:, kt, ib * IB:(ib + 1) * IB],
                        start=(kt == 0), stop=False)

            def stepB(cell=cell, woff=woff, ib=ib, dst=dst):
                for kt in range(4, NKT):
                    nc.tensor.matmul(
                        cell["ps"],
                        lhsT=wqkv_sb[:, kt, woff:woff + 128],
                        rhs=xT_sb[:, kt, ib * IB:(ib + 1) * IB],
                        start=False, stop=(kt == NKT - 1))
                nc.vector.tensor_copy(
                    out=dst[:, ib * IB:(ib + 1) * IB], in_=cell["ps"])

            return [stepA, stepB]

        def qk_steps(p):
            steps = []
            for qk in (0, 1):
                for ib in range(NIB):
                    steps += qk_steps_one(p, qk, ib)
            return steps

        def emit_v(jt):
            ps = psum.tile([128, CPC], f32, tag="qkvps")
            for kt in range(NKT):
                nc.tensor.matmul(
                    ps,
                    lhsT=xT_sb[:, kt, jt * 128:(jt + 1) * 128],
                    rhs=wqkv_sb[:, kt, 2 * CPC:3 * CPC],
                    start=(kt == 0), stop=(kt == NKT - 1))
            nc.vector.tensor_copy(out=v_sb[:, jt, :], in_=ps)

        oT_all = {}

        def outproj_steps(ib, tags=("qkvps",)):
            steps = []
            for isub in range(4):
                for nh in range(2):
                    tg = tags[(isub * 2 + nh) % len(tags)]
                    def step(isub=isub, nh=nh, ib=ib, tg=tg):
                        ops = psum.tile([128, 512], f32, tag=tg)
                        for p in range(NPAIR):
                            nc.tensor.matmul(
                                ops,
                                lhsT=oT_all[(p, ib)][:, isub * 128:(isub + 1) * 128],
                                rhs=wout_sb[:, p, nh * 512:(nh + 1) * 512],
                                start=(p == 0), stop=(p == NPAIR - 1))
                        # bf16 staging: 2x-mode DVE copy and half the DMA bytes
                        ost = outp.tile([128, 512], bf16, tag="ost")
                        nc.vector.tensor_copy(out=ost, in_=ops)
                        nc.sync.dma_start(
                            out_d[ib * IB + isub * 128: ib * IB + (isub + 1) * 128,
                                  nh * 512:(nh + 1) * 512], ost)
                    steps.append(step)
            return steps

        # ---- attention unit ----
        # P^T for a unit lives in two half tiles (j-tiles 0-7 / 8-15), each
        # [128, 2*NJH, IB] bf16 with planes indexed 2*jt_local + head.
        # The denominator add-tree runs in-place per half; roots end at
        # plane pair 0 (jt0 for lo, jt8 for hi).
        def emit_S(p, ib, jt, sAB, lo_t, hi_t):
            isl_ = slice(ib * IB, (ib + 1) * IB)
            jsl = slice(jt * 128, (jt + 1) * 128)
            t, j = (lo_t, jt) if jt < NJH else (hi_t, jt - NJH)
            nc.tensor.matmul(
                sAB[:, 0:IB],
                lhsT=kT_t[p][0:64, jsl],
                rhs=qT_t[p][0:64, isl_],
                start=True, stop=True, tile_position=(0, 0))
            nc.tensor.matmul(
                sAB[:, IB:2 * IB],
                lhsT=kT_t[p][64:128, jsl],
                rhs=qT_t[p][64:128, isl_],
                start=True, stop=True, tile_position=(64, 0))
            nc.scalar.activation(
                t[:, 2 * j:2 * j + 2, :].rearrange("p a b -> p (a b)"),
                sAB, Exp, scale=SCALE)

        def unit(p, ib, extras, handoff=None, next_info=None):
            # handoff: (lo_tile, npre) -- S/exp for jt < npre were already
            # emitted by the previous unit (cross-unit ACT backfill)
            if handoff is None:
                lo = ppool.tile([128, 2 * NJH, IB], bf16, tag="ptlo")
                npre = 0
            else:
                lo, npre = handoff
            hi = ppool1.tile([128, 2 * NJH, IB], bf16, tag="pthi")
            oT_ps = psum.tile([128, IB], f32, tag="oT")

            def pthalf(jt):
                return (lo, jt) if jt < NJH else (hi, jt - NJH)

            def emit_pv(jt):
                t, j = pthalf(jt)
                st = (jt == 0)
                sp = (jt == NJT - 1)
                nc.tensor.matmul(
                    oT_ps[0:64, :],
                    lhsT=v_sb[:, jt, (2 * p) * DH:(2 * p + 1) * DH],
                    rhs=t[:, 2 * j, :],
                    start=st, stop=sp, tile_position=(0, 0))
                nc.tensor.matmul(
                    oT_ps[64:128, :],
                    lhsT=v_sb[:, jt, (2 * p + 1) * DH:(2 * p + 2) * DH],
                    rhs=t[:, 2 * j + 1, :],
                    start=st, stop=sp, tile_position=(0, 64))

            # paced balanced merge tree over j for the denominators: one
            # in-place [128, 1024] bf16 add per iter (depth-4, same
            # accuracy as a bulk tree, but no end-of-unit DVE burst).
            def pl(t, a, n=2):
                return t[:, a:a + n, :].rearrange("p a b -> p (a b)")

            def merge(jt_dst, jt_src):
                td, d = pthalf(jt_dst)
                ts_, sr = pthalf(jt_src)
                nc.vector.tensor_tensor(
                    pl(td, 2 * d), pl(td, 2 * d), pl(ts_, 2 * sr), add)

            msched = {2: [(0, 1)], 4: [(2, 3)], 6: [(4, 5)], 8: [(6, 7)],
                      9: [(0, 2)], 10: [(8, 9)], 11: [(4, 6)], 12: [(10, 11)],
                      13: [(0, 4)], 14: [(8, 10), (12, 13)], 15: [(14, 15)]}
            extras = dict(extras)
            # with a deep backfill (npre=8), emit the remaining own S tiles
            # early (iters 2-9) so ACT never drains; not before iter 2 --
            # iter 1's deferred dnr must read the prior unit's hi planes
            # before this unit's hi tile is first written
            shift = max(0, npre - 2)
            for jt in range(NJT):
                jts = jt + shift
                if npre <= jts < NJT:
                    sAB = psum.tile([128, 2 * IB], f32, tag="sAB")
                    emit_S(p, ib, jts, sAB, lo, hi)
                if jt >= 1:
                    emit_pv(jt - 1)
                for fn in extras.pop(jt, ()):
                    fn()
                for (a, b) in msched.get(jt, ()):
                    merge(a, b)
            nxt_lo = None
            if next_info is not None:
                np_, nib = next_info
                nxt_lo = ppool.tile([128, 2 * NJH, IB], bf16, tag="ptlo",
                                    name="ptlo_h")
                sAB_h = psum.tile([128, 2 * IB], f32, tag="sAB", name="sAB_h")
                emit_S(np_, nib, 0, sAB_h, nxt_lo, None)
            emit_pv(NJT - 1)
            merge(12, 14)
            merge(8, 12)

            # PAR tail, deferred into the next unit's j-loop: the broadcast
            # denominator matmuls (PE) accumulate lo-root + hi-root per head
            # into dn_ps with the per-head sum replicated on 64 partitions;
            # recip + single normalize follow on DVE.
            def tail_dnr(p=p, ib=ib, lo=lo, hi=hi, oT_ps=oT_ps):
                cell = {}

                def dnr_mm(cell=cell):
                    dn_ps = psum.tile([128, IB], f32, tag="qkvps", name="dnps")
                    for h in range(2):
                        nc.tensor.matmul(
                            dn_ps[h * 64:(h + 1) * 64, :],
                            lhsT=ones_sb[:, 0:64], rhs=lo[:, h, :],
                            start=True, stop=False, tile_position=(0, h * 64))
                        nc.tensor.matmul(
                            dn_ps[h * 64:(h + 1) * 64, :],
                            lhsT=ones_sb[:, 0:64], rhs=hi[:, h, :],
                            start=False, stop=True, tile_position=(0, h * 64))
                    cell["dn_ps"] = dn_ps

                def norm(cell=cell):
                    dn = spool.tile([128, IB], f32, tag="dn")
                    nc.vector.reciprocal_approx_fast(dn, cell["dn_ps"])
                    oT_sb = opool.tile([128, IB], bf16, tag="oTsb")
                    nc.vector.tensor_tensor(oT_sb, oT_ps, dn, mult)
                    oT_all[(p, ib)] = oT_sb

                return [dnr_mm, norm]

            return (nxt_lo, 1) if nxt_lo is not None else None, tail_dnr()

        # ---- minimal prologue: q-ib0, k-ib0 for pair 0 ----
        for st in qk_steps_one(0, 0, 0):
            st()
        for st in qk_steps_one(0, 1, 0):
            st()

        # ---- main sweep: pair-outer / i-block-inner ----
        # extras injected per unit:
        #   unit 0: remaining V tiles (lag-1 ahead of PV), k-ib1..3 in time
        #           for S at jt=4*ib, q-ib1 late (needed by unit 1)
        #   units 1-2: pair-0's remaining q projections
        #   p<3 units 1..3 of each pair phase: next pair's qk bursts
        #   p==3 units: previous i-block's output projection
        #   every unit: previous unit's deferred PAR tail at jt=1,2
        seq = [(p, ib) for p in range(NPAIR) for ib in range(NIB)]
        # lo tiles for units 0 and 1 pre-allocated in rotation order so the
        # cross-unit S backfill (unit 0 emits unit 1's jt 0-7) keeps the
        # ppool slot-reuse ordering safe.  NOTE: this backfill must stay a
        # one-shot for unit 0 -- a STANDING npre=8 pipeline was measured
        # 38us SLOWER because the permanent ACT backlog keeps both sAB psum
        # slots full and the in-order PE head-of-line blocks on them.
        u0_lo = ppool.tile([128, 2 * NJH, IB], bf16, tag="ptlo", name="ptlo0")
        u1_lo = ppool.tile([128, 2 * NJH, IB], bf16, tag="ptlo", name="ptlo1")
        handoff = (u0_lo, 0)
        tail = None
        for i, (p, ib) in enumerate(seq):
            extras = {}
            if i == 0:
                # v(jt) one step ahead of its PV; k-ib blocks before their
                # first S use; q-ib1 early so unit 1's S tiles can backfill
                # ACT's idle windows here (npre=8 handoff)
                extras[0] = [lambda: emit_v(0)]
                for jt in range(1, NJT):
                    extras.setdefault(jt, []).append(lambda jt=jt: emit_v(jt))
                for pos, st in zip((2, 3), qk_steps_one(0, 1, 1)):
                    extras.setdefault(pos, []).append(st)
                for pos, st in zip((3, 4), qk_steps_one(0, 0, 1)):
                    extras.setdefault(pos, []).append(st)
                for pos, st in zip((6, 7), qk_steps_one(0, 1, 2)):
                    extras.setdefault(pos, []).append(st)
                for pos, st in zip((10, 11), qk_steps_one(0, 1, 3)):
                    extras.setdefault(pos, []).append(st)
                for jp in range(8):
                    def pre(jp=jp):
                        sABp = psum.tile([128, 2 * IB], f32, tag="sAB",
                                         name="sAB_h")
                        emit_S(0, 1, jp, sABp, u1_lo, None)
                    extras.setdefault(4 + jp, []).append(pre)
            else:
                spread = []
                if i == 1:
                    spread += qk_steps_one(0, 0, 2) + qk_steps_one(0, 0, 3)
                if p + 1 < NPAIR and ib == 2:
                    spread += qk_steps(p + 1)[0:6]
                elif p + 1 < NPAIR and ib == 3:
                    spread += qk_steps(p + 1)[6:12]
                elif p >= 1 and ib == 0:
                    spread += qk_steps(p)[12:16]
                elif p == 3 and ib >= 1:
                    # front-loaded (>=3: pos 2's deferred tail produces the
                    # oT operand) so the unit's last exps run gap-free and
                    # the epilogue's norm gate clears promptly
                    spread += outproj_steps(ib - 1)
                # one step per position from jt=3, overflow doubles up late
                positions = list(range(3, NJT)) + list(range(8, NJT))
                if p == 3:
                    positions = list(range(3, 11))
                if i == 1:
                    # after the shifted S burst (iters 2-9): q-ib2/3 are only
                    # needed by the end-of-unit handoffs, and at iters 3-6
                    # they throttle S emission below ACT's drain rate
                    positions = list(range(10, NJT))
                for pos, st in zip(positions, spread):
                    extras.setdefault(pos, []).append(st)
            if tail is not None:
                for pos, st in zip((1, 2), tail):
                    extras.setdefault(pos, []).append(st)
            nxt_info = seq[i + 1] if i + 1 < len(seq) else None
            if i == 0:
                nxt_info = None  # unit 1's jt 0-7 already backfilled
            handoff, tail = unit(p, ib, extras, handoff=handoff,
                                 next_info=nxt_info)
            if i == 0:
                handoff = (u1_lo, 8)
        # epilogue: bridge the norm window -- the first outproj chain's
        # pair-0..2 matmuls don't depend on the final normalize, so emit
        # them between dnr and norm to keep PE dense (and its pstate up)
        tail[0]()
        fib = NIB - 1
        ops0 = psum.tile([128, 512], f32, tag="qkvps", name="op0")
        for p in range(NPAIR - 1):
            nc.tensor.matmul(
                ops0, lhsT=oT_all[(p, fib)][:, 0:128],
                rhs=wout_sb[:, p, 0:512], start=(p == 0), stop=False)
        tail[1]()
        nc.tensor.matmul(
            ops0, lhsT=oT_all[(3, fib)][:, 0:128],
            rhs=wout_sb[:, 3, 0:512], start=False, stop=True)
        ost0 = outp.tile([128, 512], bf16, tag="ost")
        nc.vector.tensor_copy(out=ost0, in_=ops0)
        nc.sync.dma_start(out_d[fib * IB:fib * IB + 128, 0:512], ost0)
        # sAB psum banks are free after the last exp: alternate tags so four
        # chains are in flight and PE stays dense through the tail
        for st in outproj_steps(fib, tags=("sAB", "qkvps"))[1:]:
            st()


def _prep_inputs(x, w_qkv, w_out):
    bf = ml_dtypes.bfloat16
    in_maps = []
    for c in range(8):
        b, hh = c // 2, c % 2
        xT = np.ascontiguousarray(x[b].T).astype(bf)
        q = w_qkv[:, hh * CPC:(hh + 1) * CPC]
        k = w_qkv[:, DIM + hh * CPC: DIM + (hh + 1) * CPC]
        v = w_qkv[:, 2 * DIM + hh * CPC: 2 * DIM + (hh + 1) * CPC]
        wqkv = np.ascontiguousarray(np.concatenate([q, k, v], axis=1)).astype(bf)
        wout = np.ascontiguousarray(w_out[hh * CPC:(hh + 1) * CPC, :]).astype(bf)
        in_maps.append({"xT": xT, "wqkv": wqkv, "wout": wout})
    return in_maps


def _run(x, w_qkv, w_out, b_out, trace=False):
    from concourse import bass_utils
    if "nc" not in _cache:
        _cache["nc"] = _build()
    nc = _cache["nc"]
    in_maps = _prep_inputs(x, w_qkv, w_out)
    res = bass_utils.run_bass_kernel_spmd(
        nc, in_maps, core_ids=list(range(8)), trace=trace)
    partials = [np.asarray(r["out"], dtype=np.float32) for r in res.results]
    out = np.empty((B, N, DIM), dtype=np.float32)
    for b in range(B):
        out[b] = partials[2 * b] + partials[2 * b + 1] + b_out.astype(np.float32)
    return out, res


def kernel(x, w_qkv, w_out, b_out):
    x = np.asarray(x, dtype=np.float32)
    w_qkv = np.asarray(w_qkv, dtype=np.float32)
    w_out = np.asarray(w_out, dtype=np.float32)
    b_out = np.asarray(b_out, dtype=np.float32)
    out, _ = _run(x, w_qkv, w_out, b_out, trace=False)
    return out
